# revision 14
# baseline (speedup 1.0000x reference)
"""CapsuleLayer routing kernel for 8 Trainium2 NeuronCores.

Problem (full shapes): x [B=32, N=2048, IC=16] fp32,
route_weights [N=2048, K=32, IC=16, OC=32] fp32.
  priors = einsum('bni,nkio->bnko', x, W)
  3 routing iterations (softmax over K, weighted sum over N, squash)
  output = squash(s2) shaped [B, 1, K, 1, OC].

Sharding: N (nodes) sharded 8 ways (256 nodes/core); per-core W shard
(bf16, 8.4MB) is SBUF-resident; priors are recomputed on the PE each
routing pass.  Cross-core traffic: one bf16 AllReduce of s [B, K*OC]
(64KB) per non-final iteration; the final iteration's local fp32 s2
partial is DMAed out and the host sums the 8 partials + applies squash
(gather/unshard of the sum-sharded result).

Structure vs the original baseline:
  - free-dim layout is (o,k) instead of (k,o): exp(L) stays [128, 4*K]
    (no o-expansion on the scalar engine); the wp mul broadcasts exp
    over the OUTER o axis, keeping DVE 2x mode (innermost k stride 1).
  - supertiles of 16 nodes (4 groups): halves DVE/ACT instruction count.
  - zblk built on ACT via activation(Copy, scale=1/Z) per group.
  - all inputs pre-transposed on host -> contiguous [128, F] DMAs.
  - software pipelining: pp/psb/tt of supertile st+1 are issued between
    the exp and the Z/wp stages of supertile st, so the ACT queue runs
    exp before the next psb copies and DVE has independent work while
    ACT finishes exp.
  - tiny warmup AllReduce issued first absorbs the CC-stack cold start.

Engine balance per routing pass (measured): DVE ~86-89% busy (tt mul,
o-reduction tree, wp mul -- the hard wall), ACT ~57-69% (psb copies,
exp, zblk), PE ~52-57% (priors + s matmuls), gpsimd ~3% (CC/DMA
triggers only -- elementwise offload to gpsimd measurably LOSES time
via the shared DVE/gpsimd SBUF port).
"""

import numpy as np
import ml_dtypes

B, NLOC, K, IC, OC = 32, 256, 32, 16, 32
NCORES = 8
N = NLOC * NCORES
KO = K * OC            # 1024
NT = NLOC // 8         # 32 sub-tiles of 8 nodes
NST = NLOC // 16       # 16 supertiles of 16 nodes (4 groups of 4)
NGRP = NLOC // 4       # 64 groups of 4 nodes

GP_TREE = False        # tree level-1 on gpsimd (measured: net loss)

_CACHE = {}


def _build_bass():
    import concourse.bass as bass
    import concourse.mybir as mybir
    from concourse import bacc, tile

    dt = mybir.dt
    AF = mybir.ActivationFunctionType
    ALU = mybir.AluOpType

    nc = bacc.Bacc("TRN2", target_bir_lowering=False)

    wmov_d = nc.declare_dram_parameter("wmov", [128, NT * KO], dt.bfloat16, isOutput=False)
    xblk_d = nc.declare_dram_parameter("xblk", [128, NT * 128], dt.bfloat16, isOutput=False)
    xall_d = nc.declare_dram_parameter("xall", [128, NT * B], dt.bfloat16, isOutput=False)
    ones_d = nc.declare_dram_parameter("onesblk", [128, B], dt.bfloat16, isOutput=False)
    vout_d = nc.declare_dram_parameter("vout", [B, KO], dt.float32, isOutput=True)

    groups = [list(range(NCORES))]

    with tile.TileContext(nc) as tc:
        with (
            tc.tile_pool(name="wsb", bufs=1) as wpool,
            tc.tile_pool(name="persist", bufs=1) as ppool,
            tc.tile_pool(name="ltiles", bufs=NST) as lpool,
            tc.tile_pool(name="psb", bufs=5) as psb_pool,
            tc.tile_pool(name="tsb", bufs=2) as t_pool,
            tc.tile_pool(name="tree", bufs=2) as u_pool,
            tc.tile_pool(name="wp", bufs=2) as wp_pool,
            tc.tile_pool(name="eexp", bufs=3) as e_pool,
            tc.tile_pool(name="sm", bufs=4) as sm_pool,
            tc.tile_pool(name="vv", bufs=1) as v_pool,
            tc.tile_pool(name="ppsum", bufs=3, space="PSUM") as ppsum_pool,
            tc.tile_pool(name="spsum", bufs=1, space="PSUM") as spsum_pool,
            tc.tile_pool(name="dram", bufs=4, space="DRAM") as dram_pool,
        ):
            wsb = wpool.tile([128, NT * KO], dt.bfloat16, tag="wsb")
            xblk = ppool.tile([128, NT * 128], dt.bfloat16, tag="xblk")
            xall = ppool.tile([128, NT * B], dt.bfloat16, tag="xall")
            onesblk = ppool.tile([128, B], dt.bfloat16, tag="ones")

            # CC warmup: tiny AllReduce issued first, overlapping the
            # input loads, absorbs most of the collective stack cold start.
            warm_in = dram_pool.tile([1, 4], dt.float32, tag="warmin")
            warm_out = dram_pool.tile([1, 4], dt.float32, tag="warmout")
            warm_sb = v_pool.tile([1, 4], dt.float32, tag="warmsb")
            nc.vector.memset(warm_sb[:], 0.0)
            nc.gpsimd.dma_start(out=warm_in[:], in_=warm_sb[:])
            nc.gpsimd.collective_compute(
                "AllReduce", ALU.add, replica_groups=groups,
                ins=[warm_in.opt()], outs=[warm_out.opt()],
            )

            # contiguous input loads, chunked across queues
            for c in range(8):
                f0 = c * (NT * KO // 8)
                f1 = (c + 1) * (NT * KO // 8)
                nc.sync.dma_start(out=wsb[:, f0:f1], in_=wmov_d[:, f0:f1])
            for c in range(2):
                f0 = c * (NT * 64)
                f1 = (c + 1) * (NT * 64)
                nc.sync.dma_start(out=xblk[:, f0:f1], in_=xblk_d[:, f0:f1])
            nc.sync.dma_start(out=xall[:], in_=xall_d[:])
            nc.sync.dma_start(out=onesblk[:], in_=ones_d[:])

            # persistent logits tiles, one [128(4n,32b), 4*K] per supertile
            ltiles = [lpool.tile([128, 4 * K], dt.float32, tag="L", name=f"L{t}")
                      for t in range(NST)]

            def allreduce_squash(s_ps0, s_ps1, last, it):
                """PSUM s halves -> AllReduce(bf16) -> squash -> vrep tile.

                Free layout everywhere is (o,k): s[b, o*K + k].
                On the last iteration the local fp32 partial is DMAed out
                instead: the host sums the 8 partials and applies squash
                (gather/unshard of the sum-sharded result).
                """
                if last:
                    sfull = v_pool.tile([B, KO], dt.float32, tag="sfull32",
                                        name=f"sfull{it}", bufs=1)
                    nc.scalar.copy(out=sfull[:, 0:512], in_=s_ps0[:])
                    nc.scalar.copy(out=sfull[:, 512:1024], in_=s_ps1[:])
                    nc.gpsimd.dma_start(out=vout_d[:], in_=sfull[:])
                    return None
                sfull = v_pool.tile([B, KO], dt.bfloat16, tag="sfull",
                                    name=f"sfull{it}", bufs=2)
                nrm = v_pool.tile([B, K], dt.float32, tag="nrm",
                                  name=f"nrm{it}", bufs=3)
                nrm1 = v_pool.tile([B, K], dt.float32, tag="nrm1",
                                   name=f"nrm1{it}", bufs=3)
                den = v_pool.tile([B, K], dt.float32, tag="den",
                                  name=f"den{it}", bufs=3)
                rden = v_pool.tile([B, K], dt.float32, tag="rden",
                                   name=f"rden{it}", bufs=3)
                scal = v_pool.tile([B, K], dt.float32, tag="scal",
                                   name=f"scal{it}", bufs=3)
                nc.scalar.copy(out=sfull[:, 0:512], in_=s_ps0[:])
                nc.scalar.copy(out=sfull[:, 512:1024], in_=s_ps1[:])
                cc_in = dram_pool.tile([B, KO], dt.bfloat16, tag="ccin")
                cc_out = dram_pool.tile([B, KO], dt.bfloat16, tag="ccout")
                nc.gpsimd.dma_start(out=cc_in[:], in_=sfull[:])
                nc.gpsimd.collective_compute(
                    "AllReduce", ALU.add, replica_groups=groups,
                    ins=[cc_in.opt()], outs=[cc_out.opt()],
                )
                sred = v_pool.tile([B, KO], dt.bfloat16, tag="sred",
                                   name=f"sred{it}", bufs=2)
                nc.gpsimd.dma_start(out=sred[:], in_=cc_out[:])
                # squash: v = s * nrm/((1+nrm)*sqrt(nrm)), nrm = sum_o s^2
                sq1 = v_pool.tile([B, KO], dt.float32, tag="sq1",
                                  name=f"sq1{it}", bufs=1)
                nc.scalar.activation(out=sq1[:], in_=sred[:], func=AF.Square)
                nc.vector.reduce_sum(
                    out=nrm[:],
                    in_=sq1[:].rearrange("p (o k) -> p k o", o=OC),
                    axis=mybir.AxisListType.X,
                )
                nc.vector.tensor_scalar_add(nrm1[:], nrm[:], 1.0)
                nc.scalar.activation(out=den[:], in_=nrm[:], func=AF.Sqrt)
                nc.vector.reciprocal(rden[:], nrm1[:])
                nc.vector.tensor_mul(scal[:], den[:], rden[:])
                vbf = v_pool.tile([B, KO], dt.bfloat16, tag="vbf",
                                  name=f"vbf{it}", bufs=2)
                vrep = v_pool.tile([128, KO], dt.bfloat16, tag="vrep",
                                   name=f"vrep{it}", bufs=2)
                nc.vector.tensor_mul(
                    vbf[:].rearrange("p (o k) -> p o k", o=OC),
                    sred[:].rearrange("p (o k) -> p o k", o=OC),
                    scal[:].unsqueeze(1).broadcast_to((B, OC, K)),
                )
                for r in range(4):
                    nc.gpsimd.dma_start(
                        out=vrep[r * 32:(r + 1) * 32, :], in_=vbf[:]
                    )
                return vrep

            # ---------- pass A: s0 = (1/K) sum_n priors (direct matmul) -----
            s0a = spsum_pool.tile([B, 512], dt.float32, tag="sacc0")
            s0b = spsum_pool.tile([B, 512], dt.float32, tag="sacc1")
            for t in range(NT):
                nc.tensor.matmul(
                    out=s0a[:], lhsT=xall[:, t * B:(t + 1) * B],
                    rhs=wsb[:, t * KO:t * KO + 512],
                    start=(t == 0), stop=(t == NT - 1),
                )
                nc.tensor.matmul(
                    out=s0b[:], lhsT=xall[:, t * B:(t + 1) * B],
                    rhs=wsb[:, t * KO + 512:(t + 1) * KO],
                    start=(t == 0), stop=(t == NT - 1),
                )
            vrep = allreduce_squash(s0a, s0b, last=False, it=0)

            # ---------- passes B (iter1) and C (iter2) ----------------------
            for it in (1, 2):
                sa = spsum_pool.tile([B, 512], dt.float32, tag="sacc0")
                sb = spsum_pool.tile([B, 512], dt.float32, tag="sacc1")

                def stage_a(st, vrep=None, it=it):
                    """pp matmuls + psb copies + tt mul for supertile st."""
                    psb = psb_pool.tile([128, 4 * KO], dt.bfloat16, tag="psb",
                                        name=f"psb{it}_{st}")
                    for tsub in (0, 1):
                        t = 2 * st + tsub
                        for s in (0, 1):
                            g = 2 * tsub + s
                            pp = ppsum_pool.tile(
                                [128, KO], dt.float32, tag="pp",
                                name=f"pp{it}_{st}_{g}")
                            lhs = xblk[s * 64:(s + 1) * 64,
                                       t * 128:(t + 1) * 128]
                            for h in (0, 1):
                                nc.tensor.matmul(
                                    out=pp[:, h * 512:(h + 1) * 512], lhsT=lhs,
                                    rhs=wsb[s * 64:(s + 1) * 64,
                                            t * KO + h * 512:t * KO + (h + 1) * 512],
                                    start=True, stop=True,
                                    skip_group_check=True,
                                )
                            nc.scalar.copy(
                                out=psb[:, g * KO:(g + 1) * KO], in_=pp[:])
                    tt = t_pool.tile([128, 4 * KO], dt.bfloat16, tag="t",
                                     name=f"t{it}_{st}")
                    nc.vector.tensor_mul(
                        tt[:].rearrange("p (g f) -> p g f", g=4),
                        psb[:].rearrange("p (g f) -> p g f", g=4),
                        vrep[:].unsqueeze(1).broadcast_to((128, 4, KO)),
                    )
                    return psb, tt

                ab = stage_a(0, vrep)
                for st in range(NST):
                    psb, tt = ab
                    # tree reduce over o (outer axis of (o,k) layout)
                    t4 = tt[:].rearrange("p (g o k) -> p g o k", g=4, o=OC)
                    u1 = u_pool.tile([128, 4 * 16 * K], dt.bfloat16, tag="u1",
                                     name=f"u1_{it}_{st}")
                    u1v = u1[:].rearrange("p (g o k) -> p g o k", g=4, o=16)
                    eng1 = nc.gpsimd if GP_TREE else nc.vector
                    eng1.tensor_add(u1v, t4[:, :, 0:16, :], t4[:, :, 16:32, :])
                    u2 = u_pool.tile([128, 4 * 8 * K], dt.bfloat16, tag="u2",
                                     name=f"u2_{it}_{st}")
                    u2v = u2[:].rearrange("p (g o k) -> p g o k", g=4, o=8)
                    nc.vector.tensor_add(u2v, u1v[:, :, 0:8, :], u1v[:, :, 8:16, :])
                    u3 = u_pool.tile([128, 4 * 4 * K], dt.bfloat16, tag="u3",
                                     name=f"u3_{it}_{st}")
                    u3v = u3[:].rearrange("p (g o k) -> p g o k", g=4, o=4)
                    nc.vector.tensor_add(u3v, u2v[:, :, 0:4, :], u2v[:, :, 4:8, :])
                    u4 = u_pool.tile([128, 4 * 2 * K], dt.bfloat16, tag="u4",
                                     name=f"u4_{it}_{st}")
                    u4v = u4[:].rearrange("p (g o k) -> p g o k", g=4, o=2)
                    nc.vector.tensor_add(u4v, u3v[:, :, 0:2, :], u3v[:, :, 2:4, :])
                    lt4 = ltiles[st][:].rearrange("p (g o k) -> p g o k",
                                                  g=4, o=1)
                    if it == 1:
                        nc.vector.tensor_add(
                            lt4, u4v[:, :, 0:1, :], u4v[:, :, 1:2, :])
                    else:
                        dtmp = sm_pool.tile([128, 4 * K], dt.float32, tag="dtmp",
                                            name=f"dtmp{it}_{st}")
                        nc.vector.tensor_add(
                            dtmp[:].rearrange("p (g o k) -> p g o k", g=4, o=1),
                            u4v[:, :, 0:1, :], u4v[:, :, 1:2, :])
                        nc.vector.tensor_add(ltiles[st][:], ltiles[st][:],
                                             dtmp[:])
                    # exp over [128, 4*K] (small, no o-expansion)
                    eexp = e_pool.tile([128, 4 * K], dt.bfloat16, tag="eexp",
                                       name=f"eexp{it}_{st}")
                    nc.scalar.activation(out=eexp[:], in_=ltiles[st][:],
                                         func=AF.Exp)
                    # prefetch next supertile: issued here so the ACT queue
                    # runs exp (above) before the next psb copies, and the
                    # DVE does tt(st+1) while ACT finishes exp.
                    if st + 1 < NST:
                        ab = stage_a(st + 1, vrep)
                    # Z per group: sum_k exp -> [128, 4]; then 1/Z
                    zacc = sm_pool.tile([128, 4], dt.float32, tag="zacc",
                                        name=f"zacc{it}_{st}")
                    nc.vector.reduce_sum(
                        out=zacc[:],
                        in_=eexp[:].rearrange("p (g k) -> p g k", g=4),
                        axis=mybir.AxisListType.X,
                    )
                    zr = sm_pool.tile([128, 4], dt.float32, tag="zr",
                                      name=f"zr{it}_{st}")
                    nc.vector.reciprocal(zr[:], zacc[:])
                    # wp = exp * P (unnormalized); 1/Z folded into zblk
                    wp = wp_pool.tile([128, 4 * KO], dt.bfloat16, tag="wp",
                                      name=f"wp{it}_{st}")
                    nc.vector.tensor_mul(
                        wp[:].rearrange("p (g o k) -> p g o k", g=4, o=OC),
                        psb[:].rearrange("p (g o k) -> p g o k", g=4, o=OC),
                        eexp[:].rearrange("p (g k) -> p g k", g=4)
                            .unsqueeze(2).broadcast_to((128, 4, OC, K)),
                    )
                    for g in range(4):
                        zblk = sm_pool.tile([128, B], dt.bfloat16, tag="zblk",
                                            name=f"zblk{it}_{st}_{g}")
                        nc.scalar.activation(
                            out=zblk[:], in_=onesblk[:], func=AF.Copy,
                            scale=zr[:, g:g + 1])
                        gg = 4 * st + g
                        nc.tensor.matmul(
                            out=sa[:], lhsT=zblk[:],
                            rhs=wp[:, g * KO:g * KO + 512],
                            start=(gg == 0), stop=(gg == NGRP - 1),
                            skip_group_check=True,
                        )
                        nc.tensor.matmul(
                            out=sb[:], lhsT=zblk[:],
                            rhs=wp[:, g * KO + 512:(g + 1) * KO],
                            start=(gg == 0), stop=(gg == NGRP - 1),
                            skip_group_check=True,
                        )
                vrep = allreduce_squash(sa, sb, last=(it == 2), it=it)

    nc.compile()
    return nc


def _prep_inputs(x, route_weights):
    """Host-side shard + layout prep. Returns per-core in_maps.

    SBUF row layout (partition p = s*64 + j*16 + i, s in 2, j in 4, i in 16)
    matches between wmov/xblk/xall.  W free layout is (o,k): col = o*K + k.
    """
    bf16 = ml_dtypes.bfloat16
    xw = x.astype(np.float32)
    W = route_weights.astype(np.float32)
    in_maps = []
    for c in range(NCORES):
        n0 = c * NLOC
        xc = xw[:, n0:n0 + NLOC, :]          # [B, 256, IC]
        Wc = W[n0:n0 + NLOC]                 # [256, K, IC, OC]
        # wmov[s*64+j*16+i, t*KO + o*K + k] = W[8t+4s+j, k, i, o]
        wm = Wc.reshape(NT, 2, 4, K, IC, OC)       # [t, s, j, k, i, o]
        wm = wm.transpose(1, 2, 4, 0, 5, 3)        # [s, j, i, t, o, k]
        wmov = np.ascontiguousarray(
            wm.reshape(128, NT * KO)).astype(bf16)
        # xblk[s*64 + j*16 + i, t*128 + j'*32 + b] = x[b, 8t+4s+j, i]*(j==j')
        xg = xc.transpose(1, 2, 0).reshape(NT, 2, 4, IC, B)  # [t,s,j,i,b]
        xb = np.zeros((2, 4, IC, NT, 4, B), np.float32)      # [s,j,i,t,j',b]
        xgt = xg.transpose(1, 2, 3, 0, 4)                    # [s,j,i,t,b]
        for j in range(4):
            xb[:, j, :, :, j, :] = xgt[:, j]
        xblk = np.ascontiguousarray(
            xb.reshape(128, NT * 128)).astype(bf16)
        # xall[s*64+j*16+i, t*B + b] = x[b, n, i] / K
        xall = np.ascontiguousarray(
            (xgt / K).reshape(128, NT * B)).astype(bf16)
        # ones: delta(b,b') -- Z has no o-expansion in v2
        ones = np.zeros((128, B), np.float32)
        for j in range(4):
            ones[j * 32 + np.arange(32), np.arange(32)] = 1.0
        onesblk = ones.astype(bf16)
        in_maps.append({
            "wmov": wmov, "xblk": xblk, "xall": xall, "onesblk": onesblk,
        })
    return in_maps


def _get_nc():
    if "nc" not in _CACHE:
        _CACHE["nc"] = _build_bass()
    return _CACHE["nc"]


def kernel(x, route_weights, _trace=False, _trace_kwargs=None):
    from concourse.bass_utils import run_bass_kernel_spmd

    nc = _get_nc()
    in_maps = _prep_inputs(np.asarray(x), np.asarray(route_weights))
    res = run_bass_kernel_spmd(
        nc, in_maps, core_ids=list(range(NCORES)),
        trace=_trace, **(_trace_kwargs or {}),
    )
    # gather/unshard: vout holds each core's local s2 partial [B, (o,k)];
    # sum over cores, then squash on host.
    s = np.zeros((B, KO), np.float64)
    for r in res.results:
        s += r["vout"].astype(np.float64)
    s = s.reshape(B, OC, K).transpose(0, 2, 1)            # [B, K, OC]
    sq = np.sum(s * s, axis=-1, keepdims=True)
    v = (sq / (1.0 + sq)) * s / np.sqrt(sq)
    full = v.astype(np.float32).reshape(B, 1, K, 1, OC)
    if _trace:
        return full, res
    return full
